# revision 1
# baseline (speedup 1.0000x reference)
"""Trainium2 Bass kernel for nn_LlamaAttention_31782757990403.

Sparse (full + streaming) Llama attention block with W8A8 fake-quant
projections, distributed over 8 NeuronCores.

Sharding (v0, uniform SPMD — one NEFF, no branches):
  Core c owns kv head c (query heads 4c..4c+3).
    - QKV projection: output-channel shard (768 rows of qkv_w per core).
    - Attention: 4 units = 4 batches of head c. Streaming heads (c >= 4)
      get their sink+recent KV packed into the first 1088 positions and
      the remainder disabled via a per-kpos additive mask (-1e9).
    - Attention outputs are AllGathered (feature-major) across cores.
    - Output projection: row shard (512 out channels of o_w per core);
      host concatenates the 8 column slices.

Numerics:
  - Weight / activation int8 fake-quant reproduced exactly: quantized
    values are integers held in bf16 (exact), matmul accumulates f32.
  - Attention runs in fp16 (K, Q, V, P) with f32 PSUM accumulation and a
    constant -4 shift before exp for fp16 range safety (cancels in the
    softmax ratio). Softmax max-subtraction is skipped (scores are far
    from overflow in f32/exp).
  - attn-out quantization on device: round via f32->int32 convert (RNE,
    matches numpy/jax round).
"""

import numpy as np
import ml_dtypes

import concourse.bass as bass
import concourse.mybir as mybir
import concourse.tile as tile
from concourse import bacc, bass_utils
from concourse.masks import make_identity

dt = mybir.dt
AF = mybir.ActivationFunctionType
ALU = mybir.AluOpType
AX = mybir.AxisListType

NH, NKV, HD, HID = 32, 8, 128, 4096
BSZ, QLEN, PLEN = 4, 16, 4096
TOK = BSZ * QLEN                      # 64
G = NH // NKV                         # 4 query heads per kv head
N_CORES = 8
QKV_ROWS = G * HD + 2 * HD            # 768 rows of qkv_w per core
OW_ROWS = HID // N_CORES              # 512 o_w rows per core
NCH = PLEN // HD                      # 32 past-kv chunks of 128
SCL = float(1.0 / np.sqrt(np.float32(HD)))   # 1/sqrt(128)
SHIFT = -4.0                          # exp stability shift (cancels)
NEG = -1.0e9

_prog_cache = {}


def _build_program():
    nc = bacc.Bacc("TRN2", target_bir_lowering=False, debug=False,
                   num_devices=N_CORES)
    f32, f16, bf16, i32 = dt.float32, dt.float16, dt.bfloat16, dt.int32

    def inp(name, shape, d):
        return nc.dram_tensor(name, shape, d, kind="ExternalInput").ap()

    xqT = inp("xqT", [HID, TOK], bf16)
    xs = inp("xs", [TOK, 1], f32)
    wqT = inp("wqT", [HID, QKV_ROWS], bf16)
    wss = inp("wss", [1, QKV_ROWS], f32)
    cosT = inp("cosT", [TOK, 64], f32)
    sinT = inp("sinT", [TOK, 64], f32)
    kT = inp("kT", [BSZ, HD, PLEN], f16)
    vv = inp("vv", [BSZ, PLEN, HD], f16)
    maskv = inp("maskv", [HD, NCH], f32)
    newmask = inp("newmask", [QLEN, TOK], f32)
    owT = inp("owT", [HID, OW_ROWS], bf16)
    ows = inp("ows", [1, OW_ROWS], f32)
    out_ap = nc.dram_tensor("out_slice", [TOK, OW_ROWS], f32,
                            kind="ExternalOutput").ap()

    with tile.TileContext(nc, num_cores=N_CORES) as tc:
        with (
            tc.tile_pool(name="persist", bufs=1) as P1,
            tc.tile_pool(name="kt", bufs=3) as KTP,
            tc.tile_pool(name="vt", bufs=3) as VTP,
            tc.tile_pool(name="pt", bufs=4) as PTP,
            tc.tile_pool(name="work", bufs=2) as WK,
            tc.tile_pool(name="ps_proj", bufs=2, space="PSUM") as PSP,
            tc.tile_pool(name="ps_sc", bufs=2, space="PSUM") as PSS,
            tc.tile_pool(name="ps_o", bufs=2, space="PSUM") as PSO,
            tc.tile_pool(name="ps_m", bufs=2, space="PSUM") as PSM,
            tc.tile_pool(name="dram", bufs=1, space="DRAM") as DR,
        ):
            # ---------- resident loads ----------
            xqT_sb = P1.tile([HD, HID // HD, TOK], bf16)
            nc.sync.dma_start(out=xqT_sb, in_=xqT.rearrange("(c p) t -> p c t", p=HD))
            wqT_sb = P1.tile([HD, HID // HD, QKV_ROWS], bf16)
            nc.sync.dma_start(out=wqT_sb, in_=wqT.rearrange("(c p) f -> p c f", p=HD))
            owT_sb = P1.tile([HD, HID // HD, OW_ROWS], bf16)
            nc.sync.dma_start(out=owT_sb, in_=owT.rearrange("(c p) f -> p c f", p=HD))
            xs_sb = P1.tile([TOK, 1], f32)
            nc.sync.dma_start(out=xs_sb, in_=xs)
            cos_sb = P1.tile([TOK, 64], f32)
            nc.sync.dma_start(out=cos_sb, in_=cosT)
            sin_sb = P1.tile([TOK, 64], f32)
            nc.sync.dma_start(out=sin_sb, in_=sinT)
            mask_sb = P1.tile([HD, NCH], f32)
            nc.sync.dma_start(out=mask_sb, in_=maskv)
            nmask_sb = P1.tile([QLEN, TOK], f32)
            nc.sync.dma_start(out=nmask_sb, in_=newmask)
            # broadcast rows for per-feature scales
            wss_b = P1.tile([TOK, QKV_ROWS], f32)
            nc.sync.dma_start(out=wss_b, in_=bass.AP(
                tensor=wss.tensor, offset=wss.offset, ap=[[0, TOK]] + wss.ap[1:]))
            ows_b = P1.tile([TOK, OW_ROWS], f32)
            nc.sync.dma_start(out=ows_b, in_=bass.AP(
                tensor=ows.tensor, offset=ows.offset, ap=[[0, TOK]] + ows.ap[1:]))
            shift_sb = P1.tile([QLEN, 1], f32)
            nc.vector.memset(shift_sb, SHIFT)
            ident16 = P1.tile([HD, HD], f16)
            make_identity(nc, ident16)
            ident32 = P1.tile([HD, HD], f32)
            make_identity(nc, ident32)

            # ---------- QKV projection ----------
            qkv_sb = P1.tile([TOK, QKV_ROWS], f32)
            for nb in range(2):
                ncols = QKV_ROWS // 2    # 384
                ps = PSP.tile([TOK, ncols], f32, tag="proj", padded_shape=[TOK, 512])
                for kc in range(HID // HD):
                    nc.tensor.matmul(
                        ps, lhsT=xqT_sb[:, kc, :],
                        rhs=wqT_sb[:, kc, nb * ncols:(nb + 1) * ncols],
                        start=(kc == 0), stop=(kc == HID // HD - 1))
                # dequant: * xs (per token/partition) * ws (per feature)
                nc.scalar.activation(out=qkv_sb[:, nb * ncols:(nb + 1) * ncols],
                                     in_=ps, func=AF.Copy, scale=xs_sb[:, 0:1])
            nc.vector.tensor_mul(out=qkv_sb, in0=qkv_sb, in1=wss_b)

            # ---------- RoPE on q (4 heads) and k ----------
            roped = P1.tile([TOK, (G + 1) * HD], f32)
            for seg in range(G + 1):
                b0 = seg * HD
                x1 = qkv_sb[:, b0:b0 + 64]
                x2 = qkv_sb[:, b0 + 64:b0 + HD]
                t1 = WK.tile([TOK, 64], f32, tag="rope1")
                t2 = WK.tile([TOK, 64], f32, tag="rope2")
                nc.vector.tensor_mul(out=t1, in0=x1, in1=cos_sb)
                nc.vector.tensor_mul(out=t2, in0=x2, in1=sin_sb)
                nc.vector.tensor_sub(out=roped[:, b0:b0 + 64], in0=t1, in1=t2)
                nc.vector.tensor_mul(out=t1, in0=x1, in1=sin_sb)
                nc.vector.tensor_mul(out=t2, in0=x2, in1=cos_sb)
                nc.vector.tensor_add(out=roped[:, b0 + 64:b0 + HD], in0=t1, in1=t2)

            qk16 = P1.tile([TOK, (G + 1) * HD], f16)
            nc.vector.tensor_copy(out=qk16, in_=roped)
            v16f = P1.tile([TOK, HD], f16)
            nc.vector.tensor_copy(out=v16f, in_=qkv_sb[:, (G + 2) * HD - HD:])
            # per-batch v tiles at base partition 0: [16 tok, 4 batch, 129]
            v16 = P1.tile([QLEN, BSZ, HD + 1], f16)
            for b in range(BSZ):
                nc.sync.dma_start(out=v16[:, b, 0:HD],
                                  in_=v16f[b * QLEN:(b + 1) * QLEN, :])
            nc.vector.memset(v16[:, :, HD:HD + 1], 1.0)

            # transpose q heads + k: [64, 128] -> [128, 64]
            qT_sb = P1.tile([HD, G + 1, TOK], f16)
            for seg in range(G + 1):
                pst = PSM.tile([HD, TOK], f16, tag="misc")
                nc.tensor.transpose(pst, qk16[:, seg * HD:(seg + 1) * HD],
                                    ident16[0:TOK, 0:TOK])
                nc.vector.tensor_copy(out=qT_sb[:, seg, :], in_=pst)

            # ---------- attention units (4 batches of this core's kv head) ----
            contrib = DR.tile([G * HD, TOK], f32)
            gathered = DR.tile([HID, TOK], f32, addr_space="Shared")

            for b in range(BSZ):
                kt_t = KTP.tile([HD, PLEN], f16)
                nc.sync.dma_start(out=kt_t, in_=kT[b])
                v_t = VTP.tile([HD, NCH, HD + 1], f16)
                nc.sync.dma_start(out=v_t[:, :, 0:HD],
                                  in_=vv[b].rearrange("(c p) d -> p c d", p=HD))
                nc.vector.memset(v_t[:, :, HD:HD + 1], 1.0)

                qt_u = qT_sb[:, 0:G, b * QLEN:(b + 1) * QLEN]   # [128, 4, 16]
                o_ps = PSO.tile([TOK, HD + 1], f32)
                for kc in range(NCH):
                    s_ps = PSS.tile([HD, TOK], f32, tag="sc")
                    nc.tensor.matmul(s_ps, lhsT=kt_t[:, kc * HD:(kc + 1) * HD],
                                     rhs=qt_u, start=True, stop=True)
                    p_t = PTP.tile([HD, TOK], f16, tag="pt")
                    nc.scalar.activation(out=p_t, in_=s_ps, func=AF.Exp,
                                         scale=SCL, bias=mask_sb[:, kc:kc + 1])
                    nc.tensor.matmul(o_ps, lhsT=p_t, rhs=v_t[:, kc, :],
                                     start=(kc == 0), stop=False)
                # new-token chunk
                s_ps = PSS.tile([QLEN, TOK], f32, tag="sc")
                nc.tensor.matmul(s_ps, lhsT=qT_sb[:, G, b * QLEN:(b + 1) * QLEN],
                                 rhs=qt_u, start=True, stop=True)
                nc.vector.tensor_add(out=s_ps, in0=s_ps, in1=nmask_sb)
                p_t = PTP.tile([QLEN, TOK], f16, tag="pt")
                nc.scalar.activation(out=p_t, in_=s_ps, func=AF.Exp,
                                     scale=SCL, bias=shift_sb[:, 0:1])
                nc.tensor.matmul(o_ps, lhsT=p_t, rhs=v16[:, b, :],
                                 start=False, stop=True)

                # normalize by the ones-column accumulator, transpose, ship out
                rden = WK.tile([TOK, 1], f32, tag="rden")
                nc.vector.reciprocal(out=rden, in_=o_ps[:, HD:HD + 1])
                o_n = WK.tile([TOK, HD], f32, tag="on")
                nc.scalar.activation(out=o_n, in_=o_ps[:, 0:HD], func=AF.Copy,
                                     scale=rden[:, 0:1])
                ot_ps = PSM.tile([HD, TOK], f32, tag="misc")
                nc.tensor.transpose(ot_ps, o_n, ident32[0:TOK, 0:TOK])
                ot_sb = WK.tile([HD, TOK], f32, tag="ots")
                nc.vector.tensor_copy(out=ot_sb, in_=ot_ps)
                # contrib[g*128 + d, b*16 + s] = ot_sb[d, g*16 + s]
                nc.sync.dma_start(
                    out=bass.AP(tensor=contrib.tensor,
                                offset=contrib.offset + b * QLEN,
                                ap=[[TOK, HD], [HD * TOK, G], [1, QLEN]]),
                    in_=ot_sb.rearrange("p (g s) -> p g s", g=G))

            # ---------- AllGather ----------
            nc.gpsimd.collective_compute(
                "AllGather", ALU.bypass,
                replica_groups=[list(range(N_CORES))],
                ins=[contrib.opt()], outs=[gathered.opt()])

            # ---------- attn-out quantization + output projection ----------
            a_big = P1.tile([HD, HID // HD, TOK], f32)
            for qq in range(4):
                nc.sync.dma_start(
                    out=a_big[:, qq * 8:(qq + 1) * 8, :],
                    in_=gathered.rearrange("(c p) t -> p c t", p=HD)[:, qq * 8:(qq + 1) * 8, :])
            # per-token |max| via PE-transposed chunks (f32 DMA transpose n/a)
            amax = WK.tile([TOK, 1], f32, tag="amax")
            for kc in range(HID // HD):
                tp = PSM.tile([TOK, HD], f32, tag="misc")
                nc.tensor.transpose(tp, a_big[:, kc, :], ident32)
                pmax = WK.tile([TOK, 1], f32, tag="pmax")
                nc.vector.tensor_reduce(out=pmax, in_=tp, axis=AX.X, op=ALU.max,
                                        apply_absolute_value=True)
                if kc == 0:
                    nc.vector.tensor_copy(out=amax, in_=pmax)
                else:
                    nc.vector.tensor_max(out=amax, in0=amax, in1=pmax)
            s_at = P1.tile([TOK, 1], f32)
            nc.vector.tensor_scalar(out=s_at, in0=amax,
                                    scalar1=float(np.float32(1.0) / np.float32(127.0)),
                                    scalar2=1e-8, op0=ALU.mult, op1=ALU.max)
            rxs = WK.tile([TOK, 1], f32, tag="rxs")
            nc.vector.reciprocal(out=rxs, in_=s_at)
            rxs_ps = PSM.tile([1, TOK], f32, tag="misc")
            nc.tensor.transpose(rxs_ps, rxs, ident32[0:TOK, 0:TOK])
            rxs_row = WK.tile([1, TOK], f32, tag="rxsr")
            nc.vector.tensor_copy(out=rxs_row, in_=rxs_ps)
            rxs_dram = DR.tile([1, TOK], f32)
            nc.sync.dma_start(out=rxs_dram, in_=rxs_row)
            rxs_b = P1.tile([HD, TOK], f32)
            nc.sync.dma_start(out=rxs_b, in_=bass.AP(
                tensor=rxs_dram.tensor, offset=rxs_dram.offset,
                ap=[[0, HD]] + rxs_dram.ap[1:]))

            o_ps2 = PSP.tile([TOK, OW_ROWS], f32, tag="proj")
            t_i = P1.tile([HD, HID // HD, TOK], i32)
            q_at = P1.tile([HD, HID // HD, TOK], bf16)
            for kc in range(HID // HD):
                nc.vector.tensor_mul(out=t_i[:, kc, :], in0=a_big[:, kc, :],
                                     in1=rxs_b)
                nc.vector.tensor_copy(out=q_at[:, kc, :], in_=t_i[:, kc, :])
                nc.tensor.matmul(o_ps2, lhsT=q_at[:, kc, :], rhs=owT_sb[:, kc, :],
                                 start=(kc == 0), stop=(kc == HID // HD - 1))
            o_sb = P1.tile([TOK, OW_ROWS], f32)
            nc.scalar.activation(out=o_sb, in_=o_ps2, func=AF.Copy,
                                 scale=s_at[:, 0:1])
            nc.vector.tensor_mul(out=o_sb, in0=o_sb, in1=ows_b)
            nc.sync.dma_start(out=out_ap, in_=o_sb)

    nc.compile()
    return nc


def _quant_rows(w):
    s = np.maximum(np.max(np.abs(w), axis=1, keepdims=True)
                   / np.float32(127.0), np.float32(1e-8)).astype(np.float32)
    q = np.clip(np.round(w / s), -127.0, 127.0).astype(np.float32)
    return q, s[:, 0]


def kernel(x, past_k, past_v, qkv_w, o_w, q_len, num_full_kv_head,
           sink_size, recent_size):
    q_len = int(q_len); nf = int(num_full_kv_head)
    sink = int(sink_size); recent = int(recent_size)
    assert q_len == QLEN and nf == 4 and sink == 64 and recent == 1024, \
        "kernel compiled for q_len=16, nf=4, sink=64, recent=1024"
    x = np.asarray(x, np.float32)
    past_k = np.asarray(past_k, np.float32)
    past_v = np.asarray(past_v, np.float32)
    qkv_w = np.asarray(qkv_w, np.float32)
    o_w = np.asarray(o_w, np.float32)
    bf16 = ml_dtypes.bfloat16

    # ---- host prep
    xs = np.maximum(np.max(np.abs(x), axis=1, keepdims=True)
                    / np.float32(127.0), np.float32(1e-8)).astype(np.float32)
    xq = np.clip(np.round(x / xs), -127.0, 127.0).astype(np.float32)
    xqT = np.ascontiguousarray(xq.T).astype(bf16)

    wq, ws = _quant_rows(qkv_w)
    owq, ows_all = _quant_rows(o_w)

    # RoPE tables (f32 end-to-end, matching the jax reference ops)
    d_half = np.arange(0, HD, 2, dtype=np.float32) / np.float32(HD)
    inv_freq = (np.float32(1.0)
                / np.power(np.float32(10000.0), d_half)).astype(np.float32)
    pos = (PLEN + np.arange(QLEN)).astype(np.float32)
    ang = pos[:, None] * inv_freq[None, :]
    cos16 = np.cos(ang).astype(np.float32)
    sin16 = np.sin(ang).astype(np.float32)
    cosT = np.tile(cos16, (BSZ, 1))
    sinT = np.tile(sin16, (BSZ, 1))

    nm = np.full((QLEN, TOK), NEG, np.float32)
    r = np.arange(QLEN)[:, None]
    s = (np.arange(TOK) % QLEN)[None, :]
    nm[r <= s] = 0.0

    mv_full = np.full(PLEN, SHIFT, np.float32)
    sl = sink + recent                     # 1088 real streaming positions
    mv_str = np.concatenate([np.full(sl, SHIFT, np.float32),
                             np.full(PLEN - sl, NEG, np.float32)])

    in_maps = []
    for c in range(N_CORES):
        w_c = np.concatenate([
            wq[c * G * HD:(c + 1) * G * HD],
            wq[HID + c * HD:HID + (c + 1) * HD],
            wq[HID + NKV * HD + c * HD:HID + NKV * HD + (c + 1) * HD]], axis=0)
        ws_c = np.concatenate([
            ws[c * G * HD:(c + 1) * G * HD],
            ws[HID + c * HD:HID + (c + 1) * HD],
            ws[HID + NKV * HD + c * HD:HID + NKV * HD + (c + 1) * HD]])
        kT_c = np.zeros((BSZ, HD, PLEN), np.float16)
        vv_c = np.zeros((BSZ, PLEN, HD), np.float16)
        if c < nf:
            for b in range(BSZ):
                kT_c[b] = past_k[b, :, c, :].T.astype(np.float16)
                vv_c[b] = past_v[b, :, c, :].astype(np.float16)
            mv = mv_full
        else:
            for b in range(BSZ):
                kk = np.concatenate([past_k[b, :sink, c],
                                     past_k[b, PLEN - recent:, c]], axis=0)
                vvv = np.concatenate([past_v[b, :sink, c],
                                      past_v[b, PLEN - recent:, c]], axis=0)
                kT_c[b, :, :sl] = kk.T.astype(np.float16)
                vv_c[b, :sl] = vvv.astype(np.float16)
            mv = mv_str
        in_maps.append({
            "xqT": xqT, "xs": xs,
            "wqT": np.ascontiguousarray(w_c.T).astype(bf16),
            "wss": np.ascontiguousarray(ws_c[None, :]),
            "cosT": cosT, "sinT": sinT,
            "kT": kT_c, "vv": vv_c,
            "maskv": np.ascontiguousarray(mv.reshape(NCH, HD).T),
            "newmask": nm,
            "owT": np.ascontiguousarray(
                owq[c * OW_ROWS:(c + 1) * OW_ROWS].T).astype(bf16),
            "ows": np.ascontiguousarray(
                ows_all[None, c * OW_ROWS:(c + 1) * OW_ROWS]),
        })

    global _last_in_maps
    _last_in_maps = in_maps
    if "nc" not in _prog_cache:
        _prog_cache["nc"] = _build_program()
    nc = _prog_cache["nc"]

    res = bass_utils.run_bass_kernel_spmd(nc, in_maps,
                                          core_ids=list(range(N_CORES)))
    out = np.empty((TOK, HID), np.float32)
    for c in range(N_CORES):
        out[:, c * OW_ROWS:(c + 1) * OW_ROWS] = res.results[c]["out_slice"]
    return out

